# revision 15
# baseline (speedup 1.0000x reference)
"""EarthAttention3D Trainium2 Bass kernel (8 NeuronCores, window-parallel).

930 windows padded to 936 = 8*117; each core runs 117 windows.

Per window (N=144 tokens, C=192, H=6 heads, hd=32), all matmuls bf16:
  qk^T  : PE, W1-qk columns stationary over host-pretransposed x^T (K=C+1,
          ones row carries the qkv bias; q columns pre-scaled by hd^-0.5).
          M-tiles of 64 so every head's q^T/k^T lands at partition base
          0 or 32 -> S matmuls use only row groups 0/1. Evictions are
          interleaved piecewise (DVE for bank0, ScalarE for bank1) so S
          matmuls start as soon as their operand pieces land.
  v     : PE, x^T stationary over W1-v columns (v bias folded into proj bias
          via the softmax row-sum identity). Token tails (rows 128:144) of
          all 4 windows of a group are computed by one strided-lhsT matmul
          pair into gps[:, 0:192] (pad rows hold garbage, never read).
  S^T_h : PE, K=32 row-tiled 2-way (even heads group 0, odd heads group 1).
          PSUM bank layout keeps each row group in its own bank. 16-row
          token tails of 4 consecutive windows are stacked at 32-aligned
          partition offsets of group-persistent PSUM/SBUF tiles so tail
          elementwise ops amortize 4x.
  attn^T = exp(S^T) * (exp(mask)*exp(bias))^T, exp on ScalarE split per
          bank (PSUM->SBUF bf16), the precomputed multiplicative EM term
          applied on GpSimd (bodies) / VectorE (group tails) -- off the
          critical PSUM-bank cycle.
  PV    : PE, lhsT=attn^T, rhs=[v_h|1] per head (ones column -> row sums).
          Tail-K matmuls read the group att tile directly at partition base
          32j with explicit tile_position; normalization (per-partition
          reciprocal of the sums column) is fused into the PSUM eviction on
          VectorE.
  proj  : attn_out + ones col -> DMA-xbar transpose -> PE with Wp=[proj_w^T;
          pb + proj_w@bv]. The 16-row token tails of the 4 windows are
          projected by one strided-lhsT matmul pair per group.

The emission order per pipeline slot is tuned so each engine's in-order
queue has no priority inversions: the serial per-window cycle
qk -> evict -> S -> exp -> (bank WAR) -> next qk is the span floor, and
PV/proj/v matmuls fill the PE during the eviction/exp latencies.
"""

import sys

import numpy as np

sys.path.insert(0, "/opt/trn_rl_repo")

import ml_dtypes

DIM = 192
H = 6
HD = 32
WINDOW = (2, 6, 12)
N = 144
B_WIN = 930
NCORES = 8
WPC = 117
PADB = NCORES * WPC
KAUG = DIM + 1  # 193
O_QK = 384
BF = ml_dtypes.bfloat16

# W1 qk column order: M-tiles of 64: [q0 q1 | k0 k1 | q2 q3 | k2 k3 | q4 q5 |
# k4 k5]; head h sits at partition base 32*(h%2) of its M-tile.
_W1_OFF = {("q", 0): 0, ("q", 1): 32, ("k", 0): 64, ("k", 1): 96,
           ("q", 2): 128, ("q", 3): 160, ("k", 2): 192, ("k", 3): 224,
           ("q", 4): 256, ("q", 5): 288, ("k", 4): 320, ("k", 5): 352}
# qk/S/st shared psum tile columns (4 banks of 512 f32):
#   banks 0-1 ([0:432] and [512:944]): qk M-tiles, then reused for S bodies
#   banks 2-3 (1024+...): S tails, group-persistent
_MT_COL = [0, 144, 288, 512, 656, 800]  # qk M-tile -> psum col
_QC = [0, 0, 288, 288, 576, 576]  # q_h col in evicted sbQK (64, 864)
_KC = [144, 144, 432, 432, 720, 720]
_SCOL = [0, 512, 144, 656, 288, 800]  # S^T_h psum col (bank = h%2)
_ACOL = [0, 432, 144, 576, 288, 720]  # head col block in compact attn sbuf
_HORD = [0, 2, 4, 1, 3, 5]  # head order of the compact attn blocks


def _pos_index():
    wz, wh, ww = WINDOW
    coords = np.stack(
        np.meshgrid(np.arange(wz), np.arange(wh), np.arange(ww), indexing="ij")
    )
    flat = coords.reshape(3, -1)
    rel = flat[:, :, None] - flat[:, None, :]
    rel = np.transpose(rel, (1, 2, 0)).copy()
    rel[:, :, 2] += ww - 1
    rel[:, :, 1] *= 2 * ww - 1
    rel[:, :, 0] *= (2 * ww - 1) * wh * wh
    return rel.sum(-1)


POS_INDEX = _pos_index()


def _host_inputs(x, mask, qkv_w, qkv_b, proj_w, proj_b, bias_table):
    scale = float(HD) ** -0.5
    qkv_w = np.asarray(qkv_w, np.float32)
    qkv_b = np.asarray(qkv_b, np.float32)
    proj_w = np.asarray(proj_w, np.float32)
    proj_b = np.asarray(proj_b, np.float32)

    wq, wk, wv = qkv_w[0:DIM] * scale, qkv_w[DIM : 2 * DIM], qkv_w[2 * DIM :]
    bq, bk, bv = qkv_b[0:DIM] * scale, qkv_b[DIM : 2 * DIM], qkv_b[2 * DIM :]

    w1 = np.zeros((KAUG, 576), np.float32)
    for h in range(H):
        qo, ko = _W1_OFF[("q", h)], _W1_OFF[("k", h)]
        w1[0:DIM, qo : qo + HD] = wq[HD * h : HD * h + HD].T
        w1[DIM, qo : qo + HD] = bq[HD * h : HD * h + HD]
        w1[0:DIM, ko : ko + HD] = wk[HD * h : HD * h + HD].T
        w1[DIM, ko : ko + HD] = bk[HD * h : HD * h + HD]
    w1[0:DIM, O_QK:576] = wv.T
    w1 = np.ascontiguousarray(w1.astype(BF))

    wp = np.zeros((KAUG, DIM), np.float32)
    wp[0:DIM] = proj_w.T
    wp[DIM] = proj_b + proj_w @ bv
    wp = np.ascontiguousarray(wp.astype(BF))

    meantab = np.asarray(bias_table, np.float32).mean(axis=1)  # (3312, 6)
    bias3 = meantab[POS_INDEX.reshape(-1)].reshape(N, N, H)  # [n, m, h]
    ebt3 = np.exp(bias3.transpose(1, 2, 0))  # [m, h, n]
    ebt3 = np.ascontiguousarray(ebt3[:, _HORD, :])  # head-order permuted

    xp = np.zeros((PADB, N, DIM), np.float32)
    xp[:B_WIN] = x
    xt = np.ones((PADB, KAUG, N), np.float32)
    xt[:, 0:DIM, :] = xp.transpose(0, 2, 1)
    xt = xt.astype(BF).reshape(NCORES, WPC, KAUG, N)
    NG = (WPC + 3) // 4
    GP = NG * 4
    tail_rows = (np.arange(4)[:, None] * 32 + np.arange(16)[None]).ravel()

    mp = np.zeros((PADB, N, N), np.float32)
    mp[:B_WIN] = mask
    emt = np.exp(mp.transpose(0, 2, 1))  # [B, m, n]
    emb = (emt[:, :, None, :] * ebt3[None]).reshape(PADB, N, H * N)
    emb = emb.astype(BF).reshape(NCORES, WPC, N, H * N)

    HN = H * N
    in_maps = []
    for c in range(NCORES):
        emb_p = np.ones((GP, N, HN), BF)
        emb_p[:WPC] = emb[c]
        body = emb_p[:, 0:128].reshape(NG, 4, 128, HN)
        body = body.transpose(0, 2, 1, 3).reshape(NG, 128, 4 * HN)
        tails = np.ones((NG, 128, HN), BF)
        tails[:, tail_rows] = emb_p[:, 128:N].reshape(NG, 64, HN)
        emc = np.ascontiguousarray(np.concatenate([body, tails], axis=2))

        xt_p = np.ones((GP, KAUG, N), BF)
        xt_p[:WPC] = xt[c]
        xc = np.ones((NG, 128, 4 * N + 16), BF)
        xc[:, :, 0 : 4 * N] = (
            xt_p[:, 0:128].reshape(NG, 4, 128, N)
            .transpose(0, 2, 1, 3).reshape(NG, 128, 4 * N)
        )
        xb = np.ones((NG, 65, 4 * N + 16), BF)
        xb[:, :, 0 : 4 * N] = (
            xt_p[:, 128:KAUG].reshape(NG, 4, 65, N)
            .transpose(0, 2, 1, 3).reshape(NG, 65, 4 * N)
        )
        in_maps.append({"xc": xc, "xb": xb, "emc": emc, "w1": w1, "wp": wp})
    return in_maps


def _strided(ap2d, start, step, count, inner=None):
    """(P, F) AP -> (P, count[, inner]) with free stride `step` from col start."""
    import concourse.bass as bass

    base = ap2d[:, start : start + 1]
    dims = [base.ap[0], [step, count]]
    if inner is not None:
        dims.append(inner)
    return bass.AP(tensor=base.tensor, offset=base.offset, ap=dims)


def _build_kernel(tc, y, xc, xb, emc, w1, wp):
    from contextlib import ExitStack

    import concourse.mybir as mybir

    nc = tc.nc
    FP32 = mybir.dt.float32
    BF16 = mybir.dt.bfloat16
    EXP = mybir.ActivationFunctionType.Exp

    ctx = ExitStack()
    const = ctx.enter_context(tc.tile_pool(name="const", bufs=1))
    xin = ctx.enter_context(tc.tile_pool(name="xin", bufs=4))
    qksp = ctx.enter_context(tc.tile_pool(name="qksp", bufs=5))
    attns = ctx.enter_context(tc.tile_pool(name="attns", bufs=10))
    pts = ctx.enter_context(tc.tile_pool(name="pts", bufs=2))
    ysbp = ctx.enter_context(tc.tile_pool(name="ysbp", bufs=8))
    grp = ctx.enter_context(tc.tile_pool(name="grp", bufs=2))
    psA = ctx.enter_context(tc.tile_pool(name="psA", bufs=3, space="PSUM"))
    psS = ctx.enter_context(tc.tile_pool(name="psS", bufs=1, space="PSUM"))
    psG = ctx.enter_context(tc.tile_pool(name="psG", bufs=1, space="PSUM"))

    # constants
    w1a = const.tile([128, 576], BF16)
    w1b = const.tile([65, 576], BF16)
    wpa = const.tile([128, DIM], BF16)
    wpb = const.tile([65, DIM], BF16)
    nc.sync.dma_start(out=w1a, in_=w1[0:128, :])
    nc.sync.dma_start(out=w1b, in_=w1[128:KAUG, :])
    nc.sync.dma_start(out=wpa, in_=wp[0:128, :])
    nc.sync.dma_start(out=wpb, in_=wp[128:KAUG, :])

    # group-persistent tiles: two alternating hoisted sets (double-buffered
    # across groups); memsets initialize never-matmul-written rows once.
    # gps (1 bank): vt [0:192] | pvt [192:390].
    sp = psS.tile([128, 2048], FP32)
    gps = psG.tile([128, 512], FP32)
    nc.vector.memset(sp[:, 1024:2048], 0.0)
    nc.vector.memset(gps[:, :], 1.0)
    gsets = []
    for _s in range(2):
        att = grp.tile([128, H * N + 198], BF16, tag=f"att{_s}")
        nc.vector.memset(att[:, H * N : H * N + 198], 1.0)
        aog = grp.tile([128, 1280], BF16, tag=f"aog{_s}")
        nc.vector.memset(aog[:, :], 1.0)
        rect = grp.tile([128, 8], FP32, tag=f"rect{_s}")
        vsbs = []
        for _j in range(4):
            v_sb = grp.tile([128, 198], BF16, tag=f"vsb{_s}{_j}")
            nc.vector.memset(v_sb[:, :], 1.0)
            vsbs.append(v_sb)
        gsets.append((att, aog, rect, vsbs))

    NGv = (WPC + 3) // 4

    def gsz(g):
        return min(4, WPC - 4 * g)

    st = {}

    def issue_loads(g):
        s = st.setdefault(g, {})
        s["xag"] = xin.tile([128, 4 * N + 16], BF16, tag="xag", name="xag")
        s["xbg"] = xin.tile([65, 4 * N + 16], BF16, tag="xbg", name="xbg")
        s["emg"] = xin.tile([128, 5 * 864], BF16, tag="emg", name="emg")
        nc.sync.dma_start(out=s["xag"], in_=xc[g])
        nc.sync.dma_start(out=s["xbg"], in_=xb[g])
        nc.sync.dma_start(out=s["emg"], in_=emc[g])
        s["ats"] = [None] * 4

    def emit_ring_qk(g, j):
        s = st[g]
        xag, xbg = s["xag"], s["xbg"]
        xa = xag[:, j * N : (j + 1) * N]
        xbw = xbg[:, j * N : (j + 1) * N]

        qk_sb = qksp.tile([64, 864], BF16)
        s["qk_sb"] = qk_sb
        for mt in range(2):
            col = _MT_COL[mt]
            nc.tensor.matmul(sp[0:64, col : col + N],
                             w1a[:, 64 * mt : 64 * mt + 64], xa,
                             start=True, stop=False)
            nc.tensor.matmul(sp[0:64, col : col + N],
                             w1b[:, 64 * mt : 64 * mt + 64], xbw,
                             start=False, stop=True)
        nc.vector.tensor_copy(qk_sb[:, 0:288], sp[0:64, 0:288])
        nc.tensor.matmul(sp[0:64, 288 : 288 + N],
                         w1a[:, 128:192], xa, start=True, stop=False)
        nc.tensor.matmul(sp[0:64, 288 : 288 + N],
                         w1b[:, 128:192], xbw, start=False, stop=True)
        nc.vector.tensor_copy(qk_sb[:, 288:432], sp[0:64, 288:432])
        for mt in range(3, 5):
            col = _MT_COL[mt]
            nc.tensor.matmul(sp[0:64, col : col + N],
                             w1a[:, 64 * mt : 64 * mt + 64], xa,
                             start=True, stop=False)
            nc.tensor.matmul(sp[0:64, col : col + N],
                             w1b[:, 64 * mt : 64 * mt + 64], xbw,
                             start=False, stop=True)
        nc.scalar.copy(qk_sb[:, 432:720], sp[0:64, 512:800])
        nc.tensor.matmul(sp[0:64, 800 : 800 + N],
                         w1a[:, 320:384], xa, start=True, stop=False)
        nc.tensor.matmul(sp[0:64, 800 : 800 + N],
                         w1b[:, 320:384], xbw, start=False, stop=True)
        nc.scalar.copy(qk_sb[:, 720:864], sp[0:64, 800:944])

    def emit_ring_S(g, j):
        s = st[g]
        jo = 32 * j
        xag, xbg, emg = s["xag"], s["xbg"], s["emg"]
        qk_sb = s["qk_sb"]

        at = attns.tile([128, H * N], BF16)
        for h in (0, 2, 4):
            qT = qk_sb[0:32, _QC[h] : _QC[h] + N]
            kT = qk_sb[0:32, _KC[h] : _KC[h] + N]
            nc.tensor.matmul(sp[:, _SCOL[h] : _SCOL[h] + N],
                             kT[:, 0:128], qT, start=True, stop=True,
                             tile_position=(0, 0))
        nc.scalar.activation(at[:, 0:432], sp[:, 0:432], EXP)
        for h in (1, 3, 5):
            qT = qk_sb[32:64, _QC[h] : _QC[h] + N]
            kT = qk_sb[32:64, _KC[h] : _KC[h] + N]
            nc.tensor.matmul(sp[:, _SCOL[h] : _SCOL[h] + N],
                             kT[:, 0:128], qT, start=True, stop=True,
                             tile_position=(32, 0))
        nc.scalar.activation(at[:, 432:864], sp[:, 512:944], EXP)
        nc.gpsimd.tensor_mul(at[:, 0:864], at[:, 0:864],
                             emg[:, j * 864 : (j + 1) * 864])
        s["ats"][j] = at

        for h in (0, 2, 4, 1, 3, 5):
            base = 32 * (h % 2)
            qT = qk_sb[base : base + 32, _QC[h] : _QC[h] + N]
            kT = qk_sb[base : base + 32, _KC[h] : _KC[h] + N]
            nc.tensor.matmul(
                sp[jo : jo + 16, 1024 + _SCOL[h] : 1024 + _SCOL[h] + N],
                kT[:, 128:N], qT, start=True, stop=True,
                tile_position=(base, jo))

        if j == 0:
            # v token-tails of all 4 windows in one strided-lhsT matmul pair;
            # out rows 32t..32t+16 hold window t's tail, rows 32t+16..32t+32
            # garbage (never read).
            nc.tensor.matmul(gps[:, 0:DIM],
                             _strided(xag[:, :], 128, N, 4, [1, 32]),
                             w1a[:, O_QK:576], start=True, stop=False)
            nc.tensor.matmul(gps[:, 0:DIM],
                             _strided(xbg[0:64, :], 128, N, 4, [1, 32]),
                             w1b[0:64, O_QK:576], start=False, stop=True)

    def emit_ring_v(g, j):
        s = st[g]
        att, aog, rect, vsbs = gsets[g % 2]
        xag, xbg = s["xag"], s["xbg"]
        xa = xag[:, j * N : (j + 1) * N]
        xbw = xbg[:, j * N : (j + 1) * N]
        vb = psA.tile([128, 384], FP32, tag="ps")
        nc.tensor.matmul(vb[:, 0:DIM], xa[:, 0:128], w1a[:, O_QK:576],
                         start=True, stop=False)
        nc.tensor.matmul(vb[:, 0:DIM], xbw[0:64, 0:128], w1b[0:64, O_QK:576],
                         start=False, stop=True)
        v_sb = vsbs[j]
        nc.vector.tensor_copy(
            _strided(v_sb[:, :], 0, 33, H, [1, HD]),
            vb[:, 0:DIM].rearrange("p (h d) -> p h d", h=H),
        )

    def emit_group_tail(g):
        s = st[g]
        att, aog, rect, vsbs = gsets[g % 2]
        nc.scalar.activation(att[:, 0:432], sp[:, 1024:1456], EXP)
        nc.scalar.activation(att[:, 432:864], sp[:, 1536:1968], EXP)
        nc.vector.tensor_mul(att[:, 0:864], att[:, 0:864],
                             s["emg"][:, 4 * 864 : 5 * 864])
        nc.vector.tensor_copy(
            _strided(att[:, :], H * N, 33, H, [1, HD]),
            gps[:, 0:DIM].rearrange("p (h d) -> p h d", h=H),
        )

    def emit_fill_pv(g, j):
        s = st[g]
        jo = 32 * j
        att, aog, rect, vsbs = gsets[g % 2]
        at, v_sb = s["ats"][j], vsbs[j]
        pv_a = psA.tile([128, 384], FP32, tag="ps")
        s.setdefault("pvs", [None] * 4)[j] = pv_a
        for h in range(H):
            ac = _ACOL[h]
            nc.tensor.matmul(pv_a[:, 33 * h : 33 * h + 33],
                             at[:, ac : ac + 128],
                             v_sb[:, 33 * h : 33 * h + 33],
                             start=True, stop=False, tile_position=(0, 0))
            nc.tensor.matmul(pv_a[:, 33 * h : 33 * h + 33],
                             att[jo : jo + 16, ac : ac + 128],
                             att[jo : jo + 16,
                                 H * N + 33 * h : H * N + 33 * h + 33],
                             start=False, stop=True, tile_position=(jo, 0))
            nc.tensor.matmul(
                gps[jo : jo + 16, 192 + 33 * h : 225 + 33 * h],
                at[:, ac + 128 : ac + N],
                v_sb[:, 33 * h : 33 * h + 33],
                start=True, stop=False, tile_position=(0, jo))
            nc.tensor.matmul(
                gps[jo : jo + 16, 192 + 33 * h : 225 + 33 * h],
                att[jo : jo + 16, ac + 128 : ac + N],
                att[jo : jo + 16,
                    H * N + 33 * h : H * N + 33 * h + 33],
                start=False, stop=True, tile_position=(jo, jo))

    def emit_fill_norm(g, j):
        s = st[g]
        att, aog, rect, vsbs = gsets[g % 2]
        pv_a = s["pvs"][j]
        rec = ysbp.tile([128, 8], FP32, tag="rec")
        nc.vector.reciprocal(rec[:, 0:H], _strided(pv_a[:, :], 32, 33, H))
        nc.vector.tensor_mul(
            aog[:, 256 * j : 256 * j + 192].rearrange("p (h d) -> p h d", h=H),
            _strided(pv_a[:, :], 0, 33, H, [1, HD]),
            _strided(rec[:, :], 0, 1, H, [0, HD]),
        )

    def emit_group_norm(g):
        att, aog, rect, vsbs = gsets[g % 2]
        nc.vector.reciprocal(rect[:, 0:H],
                             _strided(gps[:, :], 192 + 32, 33, H))
        nc.vector.tensor_mul(
            aog[:, 1024:1216].rearrange("p (h d) -> p h d", h=H),
            _strided(gps[:, :], 192, 33, H, [1, HD]),
            _strided(rect[:, :], 0, 1, H, [0, HD]),
        )

    def emit_transpose(g):
        s = st[g]
        att, aog, rect, vsbs = gsets[g % 2]
        ptg = pts.tile([128, 1280], BF16, tag="ptg")
        nc.sync.dma_start_transpose(
            ptg[:, :].rearrange("p (b n) -> p b n", b=10), aog[:, :])
        s["ptg"] = ptg

    def emit_fill_proj(g, j):
        s = st[g]
        ptg = s["ptg"]
        if j == 0:
            s["ysbg"] = ysbp.tile([128, 4 * DIM], FP32, tag="ysbg",
                                  name="ysbg")
        yb = psA.tile([128, 384], FP32, tag="ps")
        nc.tensor.matmul(yb[:, 0:DIM], ptg[:, 256 * j : 256 * j + 128],
                         wpa, start=True, stop=False)
        nc.tensor.matmul(yb[:, 0:DIM],
                         ptg[0:65, 256 * j + 128 : 256 * j + 256], wpb,
                         start=False, stop=True)
        if j == gsz(g) - 1:
            # token-tail projection of all 4 windows: one strided-lhsT
            # matmul pair, out rows 16t..16t+16 = window t's tail.
            nc.tensor.matmul(yb[0:64, DIM : 2 * DIM],
                             _strided(ptg[:, :], 1024, 32, 4, [1, 16]),
                             wpa, start=True, stop=False)
            nc.tensor.matmul(yb[0:64, DIM : 2 * DIM],
                             _strided(ptg[0:65, :], 1152, 32, 4, [1, 16]),
                             wpb, start=False, stop=True)
            s["ytg"] = ysbp.tile([64, DIM], FP32, tag="ytg", name="ytg")
            nc.vector.tensor_copy(s["ytg"][:, :], yb[0:64, DIM : 2 * DIM])
        nc.vector.tensor_copy(s["ysbg"][:, j * DIM : (j + 1) * DIM],
                              yb[:, 0:DIM])

    def emit_stores(g):
        s = st[g]
        n = gsz(g)
        w0 = 4 * g
        nc.sync.dma_start(
            out=y[w0 : w0 + n, 0:128, :].rearrange("w p o -> p w o"),
            in_=s["ysbg"][:, 0 : n * DIM].rearrange("p (w o) -> p w o", w=n))
        nc.sync.dma_start(
            out=y[w0 : w0 + n, 128:N, :],
            in_=s["ytg"][0 : 16 * n, :])
        del st[g]

    # slot-pipelined driver: ring(s) | PV+norm fills (s-6) | proj fills (s-11)
    issue_loads(0)
    for s_i in range(4 * NGv + 15):
        g, j = divmod(s_i, 4)
        if g < NGv and j == 0 and g + 1 < NGv:
            issue_loads(g + 1)
        g2, j2 = divmod(s_i - 6, 4)
        g3, j3 = divmod(s_i - 11, 4)
        if g < NGv and j < gsz(g):
            emit_ring_qk(g, j)
        if s_i >= 6 and g2 < NGv and j2 < gsz(g2):
            emit_fill_pv(g2, j2)
            emit_fill_norm(g2, j2)
            if j2 == gsz(g2) - 1:
                emit_group_norm(g2)
                emit_transpose(g2)
        if g < NGv and j < gsz(g):
            emit_ring_S(g, j)
        if s_i >= 11 and g3 < NGv and j3 < gsz(g3):
            emit_fill_proj(g3, j3)
        if g < NGv and j < gsz(g):
            emit_ring_v(g, j)
        if g < NGv and j == gsz(g) - 1:
            emit_group_tail(g)
        if s_i >= 11 and g3 < NGv and j3 == gsz(g3) - 1:
            emit_stores(g3)

    ctx.close()


_CACHE = {}


def _get_compiled():
    if "nc" in _CACHE:
        return _CACHE["nc"]
    import concourse.tile as tile
    import concourse.mybir as mybir
    from concourse import bacc

    nc = bacc.Bacc("TRN2", target_bir_lowering=False, debug=False,
                   enable_asserts=False, num_devices=NCORES)
    BF16 = mybir.dt.bfloat16
    NGg = (WPC + 3) // 4
    xc = nc.dram_tensor("xc", (NGg, 128, 4 * N + 16), BF16,
                        kind="ExternalInput").ap()
    xb = nc.dram_tensor("xb", (NGg, 65, 4 * N + 16), BF16,
                        kind="ExternalInput").ap()
    emc = nc.dram_tensor("emc", (NGg, 128, 5 * 864), BF16,
                         kind="ExternalInput").ap()
    w1 = nc.dram_tensor("w1", (KAUG, 576), BF16, kind="ExternalInput").ap()
    wp = nc.dram_tensor("wp", (KAUG, DIM), BF16, kind="ExternalInput").ap()
    y = nc.dram_tensor("y", (WPC, N, DIM), mybir.dt.float32,
                       kind="ExternalOutput").ap()
    with tile.TileContext(nc) as tc:
        _build_kernel(tc, y, xc, xb, emc, w1, wp)
    nc.compile()
    _CACHE["nc"] = nc
    return nc


def kernel(x, mask, qkv_w, qkv_b, proj_w, proj_b, bias_table):
    from concourse.bass_utils import run_bass_kernel_spmd

    in_maps = _host_inputs(np.asarray(x), np.asarray(mask), qkv_w, qkv_b,
                           proj_w, proj_b, bias_table)
    nc = _get_compiled()
    res = run_bass_kernel_spmd(nc, in_maps, core_ids=list(range(NCORES)))
    out = np.concatenate([r["y"] for r in res.results], axis=0)
    return np.ascontiguousarray(out[:B_WIN]).astype(np.float32)


# revision 21
# speedup vs baseline: 1.5491x; 1.5491x over previous
"""EarthAttention3D Trainium2 Bass kernel (8 NeuronCores, window-parallel).

930 windows padded to 936 = 8*117; each core runs 117 windows.

Per window (N=144 tokens, C=192, H=6 heads, hd=32), all matmuls bf16:
  qk^T  : PE, W1-qk columns stationary over host-pretransposed x^T (K=C+1,
          ones row carries the qkv bias; q columns pre-scaled by hd^-0.5).
          M-tiles of 64 so every head's q^T/k^T lands at partition base
          0 or 32 -> S matmuls use only row groups 0/1.
  v     : PE, x^T stationary over W1-v columns (v bias folded into proj bias
          via the softmax row-sum identity)
  S^T_h : PE, K=32 row-tiled 2-way (even heads group 0, odd heads group 1).
          PSUM bank layout keeps each row group in its own bank: concurrent
          row-tiled matmuls in the same bank are a fatal PSUM collision, and
          Tile's tracker cannot see PE-PE concurrency.
          16-row token tails of 4 consecutive windows are stacked at
          32-aligned partition offsets of group-persistent PSUM/SBUF tiles so
          tail elementwise ops amortize 4x.
  attn^T = exp(S^T) * exp(mask)^T * exp(bias)^T
          exp on ScalarE (PSUM->SBUF bf16), *em on VectorE (broadcast AP
          across heads), *eb on GpSimd (SBUF only).
  PV    : PE, lhsT=attn^T, rhs=[v_h|1] per head (ones column -> row sums).
          The 16-row K-tail operands are DMA-moved to partition base 0 first
          so tail-K matmuls share row group 0 with body-K and accumulate into
          the same PSUM bank safely; normalization (per-partition reciprocal
          of the sums column, broadcast per head via strided APs) is fused
          into the PSUM eviction on VectorE.
  proj  : attn_out + ones col -> DMA-xbar transpose -> PE with Wp=[proj_w^T;
          pb + proj_w@bv]

DMA instructions carry a ~625ns serialized HWDGE fixed cost, so loads and
stores are batched per 4-window group (single strided DMAs) and the tail
transposes cover all four windows at once.
"""

import sys

import numpy as np

sys.path.insert(0, "/opt/trn_rl_repo")

import ml_dtypes

DIM = 192
H = 6
HD = 32
WINDOW = (2, 6, 12)
N = 144
B_WIN = 930
NCORES = 8
WPC = 117
PADB = NCORES * WPC
KAUG = DIM + 1  # 193
O_QK = 384
BF = ml_dtypes.bfloat16

# W1 qk column order: M-tiles of 64: [q0 q1 | k0 k1 | q2 q3 | k2 k3 | q4 q5 |
# k4 k5]; head h sits at partition base 32*(h%2) of its M-tile.
_W1_OFF = {("q", 0): 0, ("q", 1): 32, ("k", 0): 64, ("k", 1): 96,
           ("q", 2): 128, ("q", 3): 160, ("k", 2): 192, ("k", 3): 224,
           ("q", 4): 256, ("q", 5): 288, ("k", 4): 320, ("k", 5): 352}
# qk/S/st shared psum tile columns (4 banks of 512 f32):
#   banks 0-1 ([0:432] and [512:944]): qk M-tiles, then reused for S bodies
#   banks 2-3 (1024+...): S tails, group-persistent
_MT_COL = [0, 144, 288, 512, 656, 800]  # qk M-tile -> psum col
_QC = [0, 0, 288, 288, 576, 576]  # q_h col in evicted sbQK (64, 864)
_KC = [144, 144, 432, 432, 720, 720]
_SCOL = [0, 512, 144, 656, 288, 800]  # S^T_h psum col (bank = h%2)
_ACOL = [0, 432, 144, 576, 288, 720]  # head col block in compact attn sbuf
_HORD = [0, 2, 4, 1, 3, 5]  # head order of the compact attn blocks


def _pos_index():
    wz, wh, ww = WINDOW
    coords = np.stack(
        np.meshgrid(np.arange(wz), np.arange(wh), np.arange(ww), indexing="ij")
    )
    flat = coords.reshape(3, -1)
    rel = flat[:, :, None] - flat[:, None, :]
    rel = np.transpose(rel, (1, 2, 0)).copy()
    rel[:, :, 2] += ww - 1
    rel[:, :, 1] *= 2 * ww - 1
    rel[:, :, 0] *= (2 * ww - 1) * wh * wh
    return rel.sum(-1)


POS_INDEX = _pos_index()


def _host_inputs(x, mask, qkv_w, qkv_b, proj_w, proj_b, bias_table):
    scale = float(HD) ** -0.5
    qkv_w = np.asarray(qkv_w, np.float32)
    qkv_b = np.asarray(qkv_b, np.float32)
    proj_w = np.asarray(proj_w, np.float32)
    proj_b = np.asarray(proj_b, np.float32)

    wq, wk, wv = qkv_w[0:DIM] * scale, qkv_w[DIM : 2 * DIM], qkv_w[2 * DIM :]
    bq, bk, bv = qkv_b[0:DIM] * scale, qkv_b[DIM : 2 * DIM], qkv_b[2 * DIM :]

    w1 = np.zeros((KAUG, 576), np.float32)
    for h in range(H):
        qo, ko = _W1_OFF[("q", h)], _W1_OFF[("k", h)]
        w1[0:DIM, qo : qo + HD] = wq[HD * h : HD * h + HD].T
        w1[DIM, qo : qo + HD] = bq[HD * h : HD * h + HD]
        w1[0:DIM, ko : ko + HD] = wk[HD * h : HD * h + HD].T
        w1[DIM, ko : ko + HD] = bk[HD * h : HD * h + HD]
    w1[0:DIM, O_QK:576] = wv.T
    w1 = np.ascontiguousarray(w1.astype(BF))

    wp = np.zeros((KAUG, DIM), np.float32)
    wp[0:DIM] = proj_w.T
    wp[DIM] = proj_b + proj_w @ bv
    wp = np.ascontiguousarray(wp.astype(BF))

    meantab = np.asarray(bias_table, np.float32).mean(axis=1)  # (3312, 6)
    bias3 = meantab[POS_INDEX.reshape(-1)].reshape(N, N, H)  # [n, m, h]
    ebt3 = np.exp(bias3.transpose(1, 2, 0))  # [m, h, n]
    ebt3 = np.ascontiguousarray(ebt3[:, _HORD, :])  # head-order permuted

    xp = np.zeros((PADB, N, DIM), np.float32)
    xp[:B_WIN] = x
    xt = np.ones((PADB, KAUG, N), np.float32)
    xt[:, 0:DIM, :] = xp.transpose(0, 2, 1)
    xt = xt.astype(BF).reshape(NCORES, WPC, KAUG, N)
    NG = (WPC + 3) // 4
    GP = NG * 4
    tail_rows = (np.arange(4)[:, None] * 32 + np.arange(16)[None]).ravel()

    mp = np.zeros((PADB, N, N), np.float32)
    mp[:B_WIN] = mask
    emt = np.exp(mp.transpose(0, 2, 1))  # [B, m, n]
    emb = (emt[:, :, None, :] * ebt3[None]).reshape(PADB, N, H * N)
    emb = emb.astype(BF).reshape(NCORES, WPC, N, H * N)

    HN = H * N
    in_maps = []
    for c in range(NCORES):
        emb_p = np.ones((GP, N, HN), BF)
        emb_p[:WPC] = emb[c]
        body = emb_p[:, 0:128].reshape(NG, 4, 128, HN)
        body = body.transpose(0, 2, 1, 3).reshape(NG, 128, 4 * HN)
        tails = np.ones((NG, 128, HN), BF)
        tails[:, tail_rows] = emb_p[:, 128:N].reshape(NG, 64, HN)
        emc = np.ascontiguousarray(np.concatenate([body, tails], axis=2))

        xt_p = np.ones((GP, KAUG, N), BF)
        xt_p[:WPC] = xt[c]
        xc = xt_p[:, 0:128].reshape(NG, 4, 128, N)
        xc = np.ascontiguousarray(xc.transpose(0, 2, 1, 3).reshape(NG, 128, 4 * N))
        xb = xt_p[:, 128:KAUG].reshape(NG, 4, 65, N)
        xb = np.ascontiguousarray(xb.transpose(0, 2, 1, 3).reshape(NG, 65, 4 * N))
        in_maps.append({"xc": xc, "xb": xb, "emc": emc, "w1": w1, "wp": wp})
    return in_maps


def _strided(ap2d, start, step, count, inner=None):
    """(P, F) AP -> (P, count[, inner]) with free stride `step` from col start."""
    import concourse.bass as bass

    base = ap2d[:, start : start + 1]
    dims = [base.ap[0], [step, count]]
    if inner is not None:
        dims.append(inner)
    return bass.AP(tensor=base.tensor, offset=base.offset, ap=dims)


def _brep(ap2d, reps):
    """(P, F) AP -> (P, reps, F) with a step-0 broadcast middle dim."""
    import concourse.bass as bass

    return bass.AP(tensor=ap2d.tensor, offset=ap2d.offset,
                   ap=[ap2d.ap[0], [0, reps], *ap2d.ap[1:]])


def _build_kernel(tc, y, xc, xb, emc, w1, wp):
    from contextlib import ExitStack

    import concourse.mybir as mybir

    nc = tc.nc
    FP32 = mybir.dt.float32
    BF16 = mybir.dt.bfloat16
    EXP = mybir.ActivationFunctionType.Exp

    ctx = ExitStack()
    const = ctx.enter_context(tc.tile_pool(name="const", bufs=1))
    xin = ctx.enter_context(tc.tile_pool(name="xin", bufs=4))
    qksp = ctx.enter_context(tc.tile_pool(name="qksp", bufs=5))
    attns = ctx.enter_context(tc.tile_pool(name="attns", bufs=10))
    pts = ctx.enter_context(tc.tile_pool(name="pts", bufs=2))
    ysbp = ctx.enter_context(tc.tile_pool(name="ysbp", bufs=8))
    tsum = ctx.enter_context(tc.tile_pool(name="tsum", bufs=8))
    embp = ctx.enter_context(tc.tile_pool(name="embp", bufs=5))
    grp = ctx.enter_context(tc.tile_pool(name="grp", bufs=2))
    psA = ctx.enter_context(tc.tile_pool(name="psA", bufs=3, space="PSUM"))
    psS = ctx.enter_context(tc.tile_pool(name="psS", bufs=1, space="PSUM"))
    psG = ctx.enter_context(tc.tile_pool(name="psG", bufs=1, space="PSUM"))

    # constants
    w1a = const.tile([128, 576], BF16)
    w1b = const.tile([65, 576], BF16)
    wpa = const.tile([128, DIM], BF16)
    wpb = const.tile([65, DIM], BF16)
    nc.sync.dma_start(out=w1a, in_=w1[0:128, :])
    nc.sync.dma_start(out=w1b, in_=w1[128:KAUG, :])
    nc.sync.dma_start(out=wpa, in_=wp[0:128, :])
    nc.sync.dma_start(out=wpb, in_=wp[128:KAUG, :])

    # group-persistent tiles: two alternating hoisted sets (double-buffered
    # across groups); memsets initialize never-matmul-written rows once.
    # gps (1 bank): vt [0:192] | pvt [192:390]; y-tails reuse [192:384] after
    # the pvt reads complete.
    sp = psS.tile([128, 2048], FP32)
    gps = psG.tile([128, 512], FP32)
    nc.vector.memset(sp[:, 1024:2048], 0.0)
    nc.vector.memset(gps[:, :], 1.0)
    gsets = []
    for _s in range(2):
        att = grp.tile([128, H * N + 198], BF16, tag=f"att{_s}")
        nc.vector.memset(att[:, H * N : H * N + 198], 1.0)
        aog = grp.tile([128, 1280], BF16, tag=f"aog{_s}")
        nc.vector.memset(aog[:, :], 1.0)
        rect = grp.tile([128, 8], FP32, tag=f"rect{_s}")
        vsbs = []
        for _j in range(4):
            v_sb = grp.tile([128, 198], BF16, tag=f"vsb{_s}{_j}")
            nc.vector.memset(v_sb[:, :], 1.0)
            vsbs.append(v_sb)
        gsets.append((att, aog, rect, vsbs))

    NGv = (WPC + 3) // 4

    def gsz(g):
        return min(4, WPC - 4 * g)

    st = {}

    def issue_loads(g):
        s = st.setdefault(g, {})
        s["xag"] = xin.tile([128, 4 * N], BF16, tag="xag", name="xag")
        s["xbg"] = xin.tile([65, 4 * N], BF16, tag="xbg", name="xbg")
        s["emg"] = xin.tile([128, 5 * 864], BF16, tag="emg", name="emg")
        nc.sync.dma_start(out=s["xag"], in_=xc[g])
        nc.sync.dma_start(out=s["xbg"], in_=xb[g])
        nc.sync.dma_start(out=s["emg"], in_=emc[g])
        s["ats"] = [None] * 4
        s["at0s"] = [None] * 4

    def emit_ring(g, j):
        s = st[g]
        jo = 32 * j
        att, aog, rect, vsbs = gsets[g % 2]
        xag, xbg, emg = s["xag"], s["xbg"], s["emg"]
        xa = xag[:, j * N : (j + 1) * N]
        xbw = xbg[:, j * N : (j + 1) * N]
        em = emg[:, j * H * N : (j + 1) * H * N]

        for mt in range(6):
            col = _MT_COL[mt]
            nc.tensor.matmul(sp[0:64, col : col + N],
                             w1a[:, 64 * mt : 64 * mt + 64], xa,
                             start=True, stop=False)
            nc.tensor.matmul(sp[0:64, col : col + N],
                             w1b[:, 64 * mt : 64 * mt + 64], xbw,
                             start=False, stop=True)
        qk_sb = qksp.tile([64, 864], BF16)
        nc.vector.tensor_copy(qk_sb[:, 0:432], sp[0:64, 0:432])
        nc.scalar.copy(qk_sb[:, 432:864], sp[0:64, 512:944])

        vb = psA.tile([128, 384], FP32, tag="ps")
        nc.tensor.matmul(vb[:, 0:DIM], xa[:, 0:128], w1a[:, O_QK:576],
                         start=True, stop=False)
        nc.tensor.matmul(vb[:, 0:DIM], xbw[0:64, 0:128], w1b[0:64, O_QK:576],
                         start=False, stop=True)
        v_sb = vsbs[j]
        nc.vector.tensor_copy(
            _strided(v_sb[:, :], 0, 33, H, [1, HD]),
            vb[:, 0:DIM].rearrange("p (h d) -> p h d", h=H),
        )
        nc.tensor.matmul(gps[jo : jo + 16, 0:DIM], xa[:, 128:N],
                         w1a[:, O_QK:576], start=True, stop=False,
                         tile_position=(0, jo))
        nc.tensor.matmul(gps[jo : jo + 16, 0:DIM], xbw[0:64, 128:N],
                         w1b[0:64, O_QK:576], start=False, stop=True,
                         tile_position=(0, jo))

        for h in (0, 2, 4, 1, 3, 5):
            base = 32 * (h % 2)
            qT = qk_sb[base : base + 32, _QC[h] : _QC[h] + N]
            kT = qk_sb[base : base + 32, _KC[h] : _KC[h] + N]
            nc.tensor.matmul(sp[:, _SCOL[h] : _SCOL[h] + N],
                             kT[:, 0:128], qT, start=True, stop=True,
                             tile_position=(base, 0))
            nc.tensor.matmul(
                sp[jo : jo + 16, 1024 + _SCOL[h] : 1024 + _SCOL[h] + N],
                kT[:, 128:N], qT, start=True, stop=True,
                tile_position=(base, jo))

        at = attns.tile([128, H * N], BF16)
        nc.scalar.activation(at[:, 0:864].rearrange("p (b n) -> p b n", b=2),
                             _strided(sp[:, :], 0, 512, 2, [1, 432]), EXP)
        nc.gpsimd.tensor_mul(at[:, 0:864], at[:, 0:864], em[:, 0:864])
        s["ats"][j] = at

    def emit_group_tail(g):
        s = st[g]
        att, aog, rect, vsbs = gsets[g % 2]
        nc.scalar.activation(att[:, 0:864].rearrange("p (b n) -> p b n", b=2),
                             _strided(sp[:, :], 1024, 512, 2, [1, 432]), EXP)
        nc.vector.tensor_mul(att[:, 0:864], att[:, 0:864],
                             s["emg"][:, 4 * 864 : 5 * 864])
        nc.vector.tensor_copy(
            _strided(att[:, :], H * N, 33, H, [1, HD]),
            gps[:, 0:DIM].rearrange("p (h d) -> p h d", h=H),
        )
        for j in range(gsz(g)):
            jo = 32 * j
            at0 = tsum.tile([16, H * N + 198], BF16, tag="at0", name="at0")
            nc.sync.dma_start(out=at0, in_=att[jo : jo + 16, :])
            s["at0s"][j] = at0

    def emit_fill_pv(g, j):
        s = st[g]
        jo = 32 * j
        att, aog, rect, vsbs = gsets[g % 2]
        at, v_sb, at0 = s["ats"][j], vsbs[j], s["at0s"][j]
        pv_a = psA.tile([128, 384], FP32, tag="ps")
        s.setdefault("pvs", [None] * 4)[j] = pv_a
        for h in range(H):
            ac = _ACOL[h]
            nc.tensor.matmul(pv_a[:, 33 * h : 33 * h + 33],
                             at[:, ac : ac + 128],
                             v_sb[:, 33 * h : 33 * h + 33],
                             start=True, stop=False, tile_position=(0, 0))
            nc.tensor.matmul(pv_a[:, 33 * h : 33 * h + 33],
                             at0[:, ac : ac + 128],
                             at0[:, H * N + 33 * h : H * N + 33 * h + 33],
                             start=False, stop=True, tile_position=(0, 0))
            nc.tensor.matmul(
                gps[jo : jo + 16, 192 + 33 * h : 225 + 33 * h],
                at[:, ac + 128 : ac + N],
                v_sb[:, 33 * h : 33 * h + 33],
                start=True, stop=False, tile_position=(0, jo))
            nc.tensor.matmul(
                gps[jo : jo + 16, 192 + 33 * h : 225 + 33 * h],
                at0[:, ac + 128 : ac + N],
                at0[:, H * N + 33 * h : H * N + 33 * h + 33],
                start=False, stop=True, tile_position=(0, jo))

    def emit_fill_norm(g, j):
        s = st[g]
        att, aog, rect, vsbs = gsets[g % 2]
        pv_a = s["pvs"][j]
        rec = ysbp.tile([128, 8], FP32, tag="rec")
        nc.vector.reciprocal(rec[:, 0:H], _strided(pv_a[:, :], 32, 33, H))
        nc.vector.tensor_mul(
            aog[:, 256 * j : 256 * j + 192].rearrange("p (h d) -> p h d", h=H),
            _strided(pv_a[:, :], 0, 33, H, [1, HD]),
            _strided(rec[:, :], 0, 1, H, [0, HD]),
        )

    def emit_group_norm(g):
        att, aog, rect, vsbs = gsets[g % 2]
        nc.vector.reciprocal(rect[:, 0:H],
                             _strided(gps[:, :], 192 + 32, 33, H))
        nc.vector.tensor_mul(
            aog[:, 1024:1216].rearrange("p (h d) -> p h d", h=H),
            _strided(gps[:, :], 192, 33, H, [1, HD]),
            _strided(rect[:, :], 0, 1, H, [0, HD]),
        )

    def emit_transpose(g):
        s = st[g]
        att, aog, rect, vsbs = gsets[g % 2]
        ptg = pts.tile([128, 1280], BF16, tag="ptg")
        nc.sync.dma_start_transpose(
            ptg[:, :].rearrange("p (b n) -> p b n", b=10), aog[:, :])
        s["ptg"] = ptg

    def emit_fill_proj(g, j):
        s = st[g]
        ptg = s["ptg"]
        if j == 0:
            s["ysbg"] = ysbp.tile([128, 4 * DIM], FP32, tag="ysbg",
                                  name="ysbg")
            s["yt2"] = ysbp.tile([16, 4 * DIM], FP32, tag="yt2", name="yt2")
        yb = psA.tile([128, 384], FP32, tag="ps")
        nc.tensor.matmul(yb[:, 0:DIM], ptg[:, 256 * j : 256 * j + 128],
                         wpa, start=True, stop=False)
        nc.tensor.matmul(yb[:, 0:DIM],
                         ptg[0:65, 256 * j + 128 : 256 * j + 256], wpb,
                         start=False, stop=True)
        nc.tensor.matmul(yb[0:16, DIM : 2 * DIM],
                         ptg[:, 1024 + 32 * j : 1024 + 32 * j + 16], wpa,
                         start=True, stop=False)
        nc.tensor.matmul(yb[0:16, DIM : 2 * DIM],
                         ptg[0:65, 1152 + 32 * j : 1152 + 32 * j + 16],
                         wpb, start=False, stop=True)
        nc.scalar.copy(s["ysbg"][:, j * DIM : (j + 1) * DIM], yb[:, 0:DIM])
        nc.vector.tensor_copy(s["yt2"][:, j * DIM : (j + 1) * DIM],
                              yb[0:16, DIM : 2 * DIM])

    def emit_stores(g):
        s = st[g]
        n = gsz(g)
        w0 = 4 * g
        nc.sync.dma_start(
            out=y[w0 : w0 + n, 0:128, :].rearrange("w p o -> p w o"),
            in_=s["ysbg"][:, 0 : n * DIM].rearrange("p (w o) -> p w o", w=n))
        nc.sync.dma_start(
            out=y[w0 : w0 + n, 128:N, :].rearrange("w p o -> p w o"),
            in_=s["yt2"][:, 0 : n * DIM].rearrange("p (w o) -> p w o", w=n))
        del st[g]

    # slot-pipelined driver: ring(s) | PV+norm fills (s-6) | proj fills (s-11)
    issue_loads(0)
    for s_i in range(4 * NGv + 15):
        g, j = divmod(s_i, 4)
        if g < NGv and j == 0 and g + 1 < NGv:
            issue_loads(g + 1)
        g2, j2 = divmod(s_i - 6, 4)
        g3, j3 = divmod(s_i - 11, 4)
        if s_i >= 6 and g2 < NGv and j2 < gsz(g2):
            emit_fill_pv(g2, j2)
        if g < NGv and j < gsz(g):
            emit_ring(g, j)
        if s_i >= 6 and g2 < NGv and j2 < gsz(g2):
            emit_fill_norm(g2, j2)
            if j2 == gsz(g2) - 1:
                emit_group_norm(g2)
                emit_transpose(g2)
        if s_i >= 11 and g3 < NGv and j3 < gsz(g3):
            emit_fill_proj(g3, j3)
        if g < NGv and j == gsz(g) - 1:
            emit_group_tail(g)
        if s_i >= 11 and g3 < NGv and j3 == gsz(g3) - 1:
            emit_stores(g3)

    ctx.close()


_CACHE = {}


def _get_compiled():
    if "nc" in _CACHE:
        return _CACHE["nc"]
    import concourse.tile as tile
    import concourse.mybir as mybir
    from concourse import bacc

    nc = bacc.Bacc("TRN2", target_bir_lowering=False, debug=False,
                   enable_asserts=False, num_devices=NCORES)
    BF16 = mybir.dt.bfloat16
    NGg = (WPC + 3) // 4
    xc = nc.dram_tensor("xc", (NGg, 128, 4 * N), BF16, kind="ExternalInput").ap()
    xb = nc.dram_tensor("xb", (NGg, 65, 4 * N), BF16, kind="ExternalInput").ap()
    emc = nc.dram_tensor("emc", (NGg, 128, 5 * 864), BF16,
                         kind="ExternalInput").ap()
    w1 = nc.dram_tensor("w1", (KAUG, 576), BF16, kind="ExternalInput").ap()
    wp = nc.dram_tensor("wp", (KAUG, DIM), BF16, kind="ExternalInput").ap()
    y = nc.dram_tensor("y", (WPC, N, DIM), mybir.dt.float32,
                       kind="ExternalOutput").ap()
    with tile.TileContext(nc) as tc:
        _build_kernel(tc, y, xc, xb, emc, w1, wp)
    nc.compile()
    _CACHE["nc"] = nc
    return nc


def kernel(x, mask, qkv_w, qkv_b, proj_w, proj_b, bias_table):
    from concourse.bass_utils import run_bass_kernel_spmd

    in_maps = _host_inputs(np.asarray(x), np.asarray(mask), qkv_w, qkv_b,
                           proj_w, proj_b, bias_table)
    nc = _get_compiled()
    res = run_bass_kernel_spmd(nc, in_maps, core_ids=list(range(NCORES)))
    out = np.concatenate([r["y"] for r in res.results], axis=0)
    return np.ascontiguousarray(out[:B_WIN]).astype(np.float32)



# revision 29
# speedup vs baseline: 1.6165x; 1.0435x over previous
"""EarthAttention3D Trainium2 Bass kernel (8 NeuronCores, window-parallel).

930 windows padded to 936 = 8*117; each core runs 117 windows.

Per window (N=144 tokens, C=192, H=6 heads, hd=32), all matmuls bf16:
  qk^T  : PE, W1-qk columns stationary over host-pretransposed x^T (K=C+1,
          ones row carries the qkv bias; q columns pre-scaled by hd^-0.5).
          M-tiles of 64 so every head's q^T/k^T lands at partition base
          0 or 32 -> S matmuls use only row groups 0/1.
  v     : PE, x^T stationary over W1-v columns (v bias folded into proj bias
          via the softmax row-sum identity)
  S^T_h : PE, K=32 row-tiled 2-way (even heads group 0, odd heads group 1).
          PSUM bank layout keeps each row group in its own bank: concurrent
          row-tiled matmuls in the same bank are a fatal PSUM collision, and
          Tile's tracker cannot see PE-PE concurrency.
          16-row token tails of 4 consecutive windows are stacked at
          32-aligned partition offsets of group-persistent PSUM/SBUF tiles so
          tail elementwise ops amortize 4x.
  attn^T = exp(S^T) * exp(mask)^T * exp(bias)^T
          exp on ScalarE (PSUM->SBUF bf16), *em on VectorE (broadcast AP
          across heads), *eb on GpSimd (SBUF only).
  PV    : PE, lhsT=attn^T, rhs=[v_h|1] per head (ones column -> row sums).
          The 16-row K-tail operands are DMA-moved to partition base 0 first
          so tail-K matmuls share row group 0 with body-K and accumulate into
          the same PSUM bank safely; normalization (per-partition reciprocal
          of the sums column, broadcast per head via strided APs) is fused
          into the PSUM eviction on VectorE.
  proj  : attn_out + ones col -> DMA-xbar transpose -> PE with Wp=[proj_w^T;
          pb + proj_w@bv]

DMA instructions carry a ~625ns serialized HWDGE fixed cost, so loads and
stores are batched per 4-window group (single strided DMAs) and the tail
transposes cover all four windows at once.
"""

import sys

import numpy as np

sys.path.insert(0, "/opt/trn_rl_repo")

import ml_dtypes

DIM = 192
H = 6
HD = 32
WINDOW = (2, 6, 12)
N = 144
B_WIN = 930
NCORES = 8
WPC = 117
PADB = NCORES * WPC
KAUG = DIM + 1  # 193
O_QK = 384
BF = ml_dtypes.bfloat16

# W1 qk column order: M-tiles of 128: [q0 q1 q2 q3 | k0 k1 k2 k3 |
# q4 q5 pad pad | k4 k5 pad pad]; head h sits at partition base _QB[h] of
# its M-tile, identical for its q and k tiles.
_W1_OFF = {("q", 0): 0, ("q", 1): 32, ("q", 2): 64, ("q", 3): 96,
           ("k", 0): 128, ("k", 1): 160, ("k", 2): 192, ("k", 3): 224,
           ("q", 4): 256, ("q", 5): 288, ("k", 4): 384, ("k", 5): 416}
O_QK2 = 512  # v block offset in the widened w1
# qk/S/st shared psum tile columns (4 banks of 512 f32):
#   banks 0-1 ([0:432] and [512:944]): qk M-tiles, then reused for S bodies
#   banks 2-3 (1024+...): S tails, group-persistent
_MT_COL = [0, 144, 512, 656]  # qk M-tile -> psum col
_QB = [0, 32, 64, 96, 0, 32]  # partition base of q_h/k_h in qk_sb
_QC = [0, 0, 0, 0, 288, 288]  # q_h col block in evicted sbQK (128, 576)
_KC = [144, 144, 144, 144, 432, 432]
_SCOL = [0, 512, 144, 656, 288, 800]  # S^T_h psum col (bank = h%2)
_ACOL = [0, 432, 144, 576, 288, 720]  # head col block in compact attn sbuf
_HORD = [0, 2, 4, 1, 3, 5]  # head order of the compact attn blocks


def _pos_index():
    wz, wh, ww = WINDOW
    coords = np.stack(
        np.meshgrid(np.arange(wz), np.arange(wh), np.arange(ww), indexing="ij")
    )
    flat = coords.reshape(3, -1)
    rel = flat[:, :, None] - flat[:, None, :]
    rel = np.transpose(rel, (1, 2, 0)).copy()
    rel[:, :, 2] += ww - 1
    rel[:, :, 1] *= 2 * ww - 1
    rel[:, :, 0] *= (2 * ww - 1) * wh * wh
    return rel.sum(-1)


POS_INDEX = _pos_index()


def _host_inputs(x, mask, qkv_w, qkv_b, proj_w, proj_b, bias_table):
    scale = float(HD) ** -0.5
    qkv_w = np.asarray(qkv_w, np.float32)
    qkv_b = np.asarray(qkv_b, np.float32)
    proj_w = np.asarray(proj_w, np.float32)
    proj_b = np.asarray(proj_b, np.float32)

    wq, wk, wv = qkv_w[0:DIM] * scale, qkv_w[DIM : 2 * DIM], qkv_w[2 * DIM :]
    bq, bk, bv = qkv_b[0:DIM] * scale, qkv_b[DIM : 2 * DIM], qkv_b[2 * DIM :]

    w1 = np.zeros((KAUG, 704), np.float32)
    for h in range(H):
        qo, ko = _W1_OFF[("q", h)], _W1_OFF[("k", h)]
        w1[0:DIM, qo : qo + HD] = wq[HD * h : HD * h + HD].T
        w1[DIM, qo : qo + HD] = bq[HD * h : HD * h + HD]
        w1[0:DIM, ko : ko + HD] = wk[HD * h : HD * h + HD].T
        w1[DIM, ko : ko + HD] = bk[HD * h : HD * h + HD]
    # pad slots of the q4q5/k4k5 tiles duplicate q0q1/k0k1 (harmless)
    w1[:, 320:384] = w1[:, 0:64]
    w1[:, 448:512] = w1[:, 128:192]
    w1[0:DIM, O_QK2:704] = wv.T
    w1 = np.ascontiguousarray(w1.astype(BF))

    wp = np.zeros((KAUG, DIM), np.float32)
    wp[0:DIM] = proj_w.T
    wp[DIM] = proj_b + proj_w @ bv
    wp = np.ascontiguousarray(wp.astype(BF))

    meantab = np.asarray(bias_table, np.float32).mean(axis=1)  # (3312, 6)
    bias3 = meantab[POS_INDEX.reshape(-1)].reshape(N, N, H)  # [n, m, h]
    ebt3 = np.exp(bias3.transpose(1, 2, 0))  # [m, h, n]
    ebt3 = np.ascontiguousarray(ebt3[:, _HORD, :])  # head-order permuted

    xp = np.zeros((PADB, N, DIM), np.float32)
    xp[:B_WIN] = x
    xt = np.ones((PADB, KAUG, N), np.float32)
    xt[:, 0:DIM, :] = xp.transpose(0, 2, 1)
    xt = xt.astype(BF).reshape(NCORES, WPC, KAUG, N)
    NG = (WPC + 3) // 4
    GP = NG * 4
    tail_rows = (np.arange(4)[:, None] * 32 + np.arange(16)[None]).ravel()

    mp = np.zeros((PADB, N, N), np.float32)
    mp[:B_WIN] = mask
    emt = np.exp(mp.transpose(0, 2, 1))  # [B, m, n]
    emb = (emt[:, :, None, :] * ebt3[None]).reshape(PADB, N, H * N)
    emb = emb.astype(BF).reshape(NCORES, WPC, N, H * N)

    HN = H * N
    in_maps = []
    for c in range(NCORES):
        emb_p = np.ones((GP, N, HN), BF)
        emb_p[:WPC] = emb[c]
        body = emb_p[:, 0:128].reshape(NG, 4, 128, HN)
        body = body.transpose(0, 2, 1, 3).reshape(NG, 128, 4 * HN)
        tails = np.ones((NG, 128, HN), BF)
        tails[:, tail_rows] = emb_p[:, 128:N].reshape(NG, 64, HN)
        emc = np.ascontiguousarray(np.concatenate([body, tails], axis=2))

        xt_p = np.ones((GP, KAUG, N), BF)
        xt_p[:WPC] = xt[c]
        xc = xt_p[:, 0:128].reshape(NG, 4, 128, N)
        xc = np.ascontiguousarray(xc.transpose(0, 2, 1, 3).reshape(NG, 128, 4 * N))
        xb = xt_p[:, 128:KAUG].reshape(NG, 4, 65, N)
        xb = np.ascontiguousarray(xb.transpose(0, 2, 1, 3).reshape(NG, 65, 4 * N))
        in_maps.append({"xc": xc, "xb": xb, "emc": emc, "w1": w1, "wp": wp})
    return in_maps


def _strided(ap2d, start, step, count, inner=None):
    """(P, F) AP -> (P, count[, inner]) with free stride `step` from col start."""
    import concourse.bass as bass

    base = ap2d[:, start : start + 1]
    dims = [base.ap[0], [step, count]]
    if inner is not None:
        dims.append(inner)
    return bass.AP(tensor=base.tensor, offset=base.offset, ap=dims)


def _brep(ap2d, reps):
    """(P, F) AP -> (P, reps, F) with a step-0 broadcast middle dim."""
    import concourse.bass as bass

    return bass.AP(tensor=ap2d.tensor, offset=ap2d.offset,
                   ap=[ap2d.ap[0], [0, reps], *ap2d.ap[1:]])


def _build_kernel(tc, y, xc, xb, emc, w1, wp):
    from contextlib import ExitStack

    import concourse.mybir as mybir

    nc = tc.nc
    FP32 = mybir.dt.float32
    BF16 = mybir.dt.bfloat16
    EXP = mybir.ActivationFunctionType.Exp

    ctx = ExitStack()
    const = ctx.enter_context(tc.tile_pool(name="const", bufs=1))
    xin = ctx.enter_context(tc.tile_pool(name="xin", bufs=4))
    qksp = ctx.enter_context(tc.tile_pool(name="qksp", bufs=5))
    attns = ctx.enter_context(tc.tile_pool(name="attns", bufs=10))
    pts = ctx.enter_context(tc.tile_pool(name="pts", bufs=2))
    ysbp = ctx.enter_context(tc.tile_pool(name="ysbp", bufs=8))
    tsum = ctx.enter_context(tc.tile_pool(name="tsum", bufs=8))
    embp = ctx.enter_context(tc.tile_pool(name="embp", bufs=5))
    grp = ctx.enter_context(tc.tile_pool(name="grp", bufs=2))
    psA = ctx.enter_context(tc.tile_pool(name="psA", bufs=3, space="PSUM"))
    psS = ctx.enter_context(tc.tile_pool(name="psS", bufs=1, space="PSUM"))
    psG = ctx.enter_context(tc.tile_pool(name="psG", bufs=1, space="PSUM"))

    # constants
    w1a = const.tile([128, 704], BF16)
    w1b = const.tile([65, 704], BF16)
    wpa = const.tile([128, DIM], BF16)
    wpb = const.tile([65, DIM], BF16)
    nc.sync.dma_start(out=w1a, in_=w1[0:128, :])
    nc.sync.dma_start(out=w1b, in_=w1[128:KAUG, :])
    nc.sync.dma_start(out=wpa, in_=wp[0:128, :])
    nc.sync.dma_start(out=wpb, in_=wp[128:KAUG, :])

    # group-persistent tiles: two alternating hoisted sets (double-buffered
    # across groups); memsets initialize never-matmul-written rows once.
    # gps (1 bank): vt [0:192] | pvt [192:390]; y-tails reuse [192:384] after
    # the pvt reads complete.
    sp = psS.tile([128, 2048], FP32)
    gps = psG.tile([128, 512], FP32)
    nc.vector.memset(sp[:, 1024:2048], 0.0)
    nc.vector.memset(gps[:, :], 1.0)
    gsets = []
    for _s in range(2):
        att = grp.tile([128, H * N + 198], BF16, tag=f"att{_s}")
        nc.vector.memset(att[:, H * N : H * N + 198], 1.0)
        aog = grp.tile([128, 1280], BF16, tag=f"aog{_s}")
        nc.vector.memset(aog[:, :], 1.0)
        rect = grp.tile([128, 8], FP32, tag=f"rect{_s}")
        vsbs = []
        for _j in range(4):
            v_sb = grp.tile([128, 198], BF16, tag=f"vsb{_s}{_j}")
            nc.vector.memset(v_sb[:, :], 1.0)
            vsbs.append(v_sb)
        gsets.append((att, aog, rect, vsbs))

    NGv = (WPC + 3) // 4

    def gsz(g):
        return min(4, WPC - 4 * g)

    st = {}

    def issue_loads(g):
        s = st.setdefault(g, {})
        s["xag"] = xin.tile([128, 4 * N], BF16, tag="xag", name="xag")
        s["xbg"] = xin.tile([65, 4 * N], BF16, tag="xbg", name="xbg")
        s["emg"] = xin.tile([128, 5 * 864], BF16, tag="emg", name="emg")
        nc.sync.dma_start(out=s["xag"], in_=xc[g])
        nc.sync.dma_start(out=s["xbg"], in_=xb[g])
        nc.sync.dma_start(out=s["emg"], in_=emc[g])
        s["ats"] = [None] * 4
        s["at0s"] = [None] * 4

    def emit_ring(g, j):
        s = st[g]
        jo = 32 * j
        att, aog, rect, vsbs = gsets[g % 2]
        xag, xbg, emg = s["xag"], s["xbg"], s["emg"]
        xa = xag[:, j * N : (j + 1) * N]
        xbw = xbg[:, j * N : (j + 1) * N]
        em = emg[:, j * H * N : (j + 1) * H * N]

        for mt in range(2):
            col = _MT_COL[mt]
            nc.tensor.matmul(sp[:, col : col + N],
                             w1a[:, 128 * mt : 128 * mt + 128], xa,
                             start=True, stop=False)
            nc.tensor.matmul(sp[:, col : col + N],
                             w1b[:, 128 * mt : 128 * mt + 128], xbw,
                             start=False, stop=True)
        qk_sb = qksp.tile([128, 576], BF16)
        nc.vector.tensor_copy(qk_sb[:, 0:288], sp[:, 0:288])
        for mt in range(2, 4):
            col = _MT_COL[mt]
            nc.tensor.matmul(sp[:, col : col + N],
                             w1a[:, 128 * mt : 128 * mt + 128], xa,
                             start=True, stop=False)
            nc.tensor.matmul(sp[:, col : col + N],
                             w1b[:, 128 * mt : 128 * mt + 128], xbw,
                             start=False, stop=True)
        nc.scalar.copy(qk_sb[:, 288:576], sp[:, 512:800])

        vb = psA.tile([128, 384], FP32, tag="ps")
        v_sb = vsbs[j]

        def s_mm(h):
            base = _QB[h]
            qT = qk_sb[base : base + 32, _QC[h] : _QC[h] + N]
            kT = qk_sb[base : base + 32, _KC[h] : _KC[h] + N]
            nc.tensor.matmul(sp[:, _SCOL[h] : _SCOL[h] + N],
                             kT[:, 0:128], qT, start=True, stop=True,
                             tile_position=(base, 0))
            nc.tensor.matmul(
                sp[jo : jo + 16, 1024 + _SCOL[h] : 1024 + _SCOL[h] + N],
                kT[:, 128:N], qT, start=True, stop=True,
                tile_position=(base, jo))

        # full-array matmuls (v / gps-v chunk 1) act as row-group barriers
        # between reuses of the same S psum bank from different row groups.
        s_mm(0)
        s_mm(1)
        nc.tensor.matmul(vb[:, 0:DIM], xa[:, 0:128], w1a[:, O_QK2:704],
                         start=True, stop=False)
        s_mm(2)
        s_mm(3)
        nc.tensor.matmul(gps[jo : jo + 16, 0:DIM], xa[:, 128:N],
                         w1a[:, O_QK2:704], start=True, stop=False,
                         tile_position=(0, jo))
        s_mm(4)
        s_mm(5)
        nc.tensor.matmul(vb[:, 0:DIM], xbw[0:64, 0:128],
                         w1b[0:64, O_QK2:704], start=False, stop=True)
        nc.tensor.matmul(gps[jo : jo + 16, 0:DIM], xbw[0:64, 128:N],
                         w1b[0:64, O_QK2:704], start=False, stop=True,
                         tile_position=(0, jo))
        nc.vector.tensor_copy(
            _strided(v_sb[:, :], 0, 33, H, [1, HD]),
            vb[:, 0:DIM].rearrange("p (h d) -> p h d", h=H),
        )

        at = attns.tile([128, H * N], BF16)
        nc.scalar.activation(at[:, 0:864].rearrange("p (b n) -> p b n", b=2),
                             _strided(sp[:, :], 0, 512, 2, [1, 432]), EXP)
        nc.gpsimd.tensor_mul(at[:, 0:864], at[:, 0:864], em[:, 0:864])
        s["ats"][j] = at

    def emit_group_tail(g):
        s = st[g]
        att, aog, rect, vsbs = gsets[g % 2]
        nc.scalar.activation(att[:, 0:864].rearrange("p (b n) -> p b n", b=2),
                             _strided(sp[:, :], 1024, 512, 2, [1, 432]), EXP)
        nc.vector.tensor_mul(att[:, 0:864], att[:, 0:864],
                             s["emg"][:, 4 * 864 : 5 * 864])
        nc.vector.tensor_copy(
            _strided(att[:, :], H * N, 33, H, [1, HD]),
            gps[:, 0:DIM].rearrange("p (h d) -> p h d", h=H),
        )
        for j in range(gsz(g)):
            jo = 32 * j
            at0 = tsum.tile([16, H * N + 198], BF16, tag="at0", name="at0")
            nc.sync.dma_start(out=at0, in_=att[jo : jo + 16, :])
            s["at0s"][j] = at0

    def emit_fill_pv(g, j):
        s = st[g]
        jo = 32 * j
        att, aog, rect, vsbs = gsets[g % 2]
        at, v_sb, at0 = s["ats"][j], vsbs[j], s["at0s"][j]
        pv_a = psA.tile([128, 384], FP32, tag="ps")
        s.setdefault("pvs", [None] * 4)[j] = pv_a
        for h in range(H):
            ac = _ACOL[h]
            nc.tensor.matmul(pv_a[:, 33 * h : 33 * h + 33],
                             at[:, ac : ac + 128],
                             v_sb[:, 33 * h : 33 * h + 33],
                             start=True, stop=False, tile_position=(0, 0))
            nc.tensor.matmul(pv_a[:, 33 * h : 33 * h + 33],
                             at0[:, ac : ac + 128],
                             at0[:, H * N + 33 * h : H * N + 33 * h + 33],
                             start=False, stop=True, tile_position=(0, 0))
            nc.tensor.matmul(
                gps[jo : jo + 16, 192 + 33 * h : 225 + 33 * h],
                at[:, ac + 128 : ac + N],
                v_sb[:, 33 * h : 33 * h + 33],
                start=True, stop=False, tile_position=(0, jo))
            nc.tensor.matmul(
                gps[jo : jo + 16, 192 + 33 * h : 225 + 33 * h],
                at0[:, ac + 128 : ac + N],
                at0[:, H * N + 33 * h : H * N + 33 * h + 33],
                start=False, stop=True, tile_position=(0, jo))

    def emit_fill_norm(g, j):
        s = st[g]
        att, aog, rect, vsbs = gsets[g % 2]
        pv_a = s["pvs"][j]
        rec = ysbp.tile([128, 8], FP32, tag="rec")
        nc.vector.reciprocal(rec[:, 0:H], _strided(pv_a[:, :], 32, 33, H))
        nc.vector.tensor_mul(
            aog[:, 256 * j : 256 * j + 192].rearrange("p (h d) -> p h d", h=H),
            _strided(pv_a[:, :], 0, 33, H, [1, HD]),
            _strided(rec[:, :], 0, 1, H, [0, HD]),
        )

    def emit_group_norm(g):
        att, aog, rect, vsbs = gsets[g % 2]
        nc.vector.reciprocal(rect[:, 0:H],
                             _strided(gps[:, :], 192 + 32, 33, H))
        nc.vector.tensor_mul(
            aog[:, 1024:1216].rearrange("p (h d) -> p h d", h=H),
            _strided(gps[:, :], 192, 33, H, [1, HD]),
            _strided(rect[:, :], 0, 1, H, [0, HD]),
        )

    def emit_transpose(g):
        s = st[g]
        att, aog, rect, vsbs = gsets[g % 2]
        ptg = pts.tile([128, 1280], BF16, tag="ptg")
        nc.sync.dma_start_transpose(
            ptg[:, :].rearrange("p (b n) -> p b n", b=10), aog[:, :])
        s["ptg"] = ptg

    def emit_fill_proj(g, j):
        s = st[g]
        ptg = s["ptg"]
        if j == 0:
            s["ysbg"] = ysbp.tile([128, 4 * DIM], FP32, tag="ysbg",
                                  name="ysbg")
            s["yt2"] = ysbp.tile([16, 4 * DIM], FP32, tag="yt2", name="yt2")
        yb = psA.tile([128, 384], FP32, tag="ps")
        nc.tensor.matmul(yb[:, 0:DIM], ptg[:, 256 * j : 256 * j + 128],
                         wpa, start=True, stop=False)
        nc.tensor.matmul(yb[:, 0:DIM],
                         ptg[0:65, 256 * j + 128 : 256 * j + 256], wpb,
                         start=False, stop=True)
        nc.tensor.matmul(yb[0:16, DIM : 2 * DIM],
                         ptg[:, 1024 + 32 * j : 1024 + 32 * j + 16], wpa,
                         start=True, stop=False)
        nc.tensor.matmul(yb[0:16, DIM : 2 * DIM],
                         ptg[0:65, 1152 + 32 * j : 1152 + 32 * j + 16],
                         wpb, start=False, stop=True)
        nc.scalar.copy(s["ysbg"][:, j * DIM : (j + 1) * DIM], yb[:, 0:DIM])
        nc.vector.tensor_copy(s["yt2"][:, j * DIM : (j + 1) * DIM],
                              yb[0:16, DIM : 2 * DIM])

    def emit_stores(g):
        s = st[g]
        n = gsz(g)
        w0 = 4 * g
        nc.sync.dma_start(
            out=y[w0 : w0 + n, 0:128, :].rearrange("w p o -> p w o"),
            in_=s["ysbg"][:, 0 : n * DIM].rearrange("p (w o) -> p w o", w=n))
        nc.sync.dma_start(
            out=y[w0 : w0 + n, 128:N, :].rearrange("w p o -> p w o"),
            in_=s["yt2"][:, 0 : n * DIM].rearrange("p (w o) -> p w o", w=n))
        del st[g]

    # slot-pipelined driver: ring(s) | PV+norm fills (s-6) | proj fills (s-11)
    issue_loads(0)
    for s_i in range(4 * NGv + 15):
        g, j = divmod(s_i, 4)
        if g < NGv and j == 0 and g + 1 < NGv:
            issue_loads(g + 1)
        g2, j2 = divmod(s_i - 6, 4)
        g3, j3 = divmod(s_i - 11, 4)
        if s_i >= 6 and g2 < NGv and j2 < gsz(g2):
            emit_fill_pv(g2, j2)
        if g < NGv and j < gsz(g):
            emit_ring(g, j)
        if s_i >= 6 and g2 < NGv and j2 < gsz(g2):
            emit_fill_norm(g2, j2)
            if j2 == gsz(g2) - 1:
                emit_group_norm(g2)
                emit_transpose(g2)
        if s_i >= 11 and g3 < NGv and j3 < gsz(g3):
            emit_fill_proj(g3, j3)
        if g < NGv and j == gsz(g) - 1:
            emit_group_tail(g)
        if s_i >= 11 and g3 < NGv and j3 == gsz(g3) - 1:
            emit_stores(g3)

    ctx.close()


_CACHE = {}


def _get_compiled():
    if "nc" in _CACHE:
        return _CACHE["nc"]
    import concourse.tile as tile
    import concourse.mybir as mybir
    from concourse import bacc

    nc = bacc.Bacc("TRN2", target_bir_lowering=False, debug=False,
                   enable_asserts=False, num_devices=NCORES)
    BF16 = mybir.dt.bfloat16
    NGg = (WPC + 3) // 4
    xc = nc.dram_tensor("xc", (NGg, 128, 4 * N), BF16, kind="ExternalInput").ap()
    xb = nc.dram_tensor("xb", (NGg, 65, 4 * N), BF16, kind="ExternalInput").ap()
    emc = nc.dram_tensor("emc", (NGg, 128, 5 * 864), BF16,
                         kind="ExternalInput").ap()
    w1 = nc.dram_tensor("w1", (KAUG, 704), BF16, kind="ExternalInput").ap()
    wp = nc.dram_tensor("wp", (KAUG, DIM), BF16, kind="ExternalInput").ap()
    y = nc.dram_tensor("y", (WPC, N, DIM), mybir.dt.float32,
                       kind="ExternalOutput").ap()
    with tile.TileContext(nc) as tc:
        _build_kernel(tc, y, xc, xb, emc, w1, wp)
    nc.compile()
    _CACHE["nc"] = nc
    return nc


def kernel(x, mask, qkv_w, qkv_b, proj_w, proj_b, bias_table):
    from concourse.bass_utils import run_bass_kernel_spmd

    in_maps = _host_inputs(np.asarray(x), np.asarray(mask), qkv_w, qkv_b,
                           proj_w, proj_b, bias_table)
    nc = _get_compiled()
    res = run_bass_kernel_spmd(nc, in_maps, core_ids=list(range(NCORES)))
    out = np.concatenate([r["y"] for r in res.results], axis=0)
    return np.ascontiguousarray(out[:B_WIN]).astype(np.float32)

